# revision 15
# baseline (speedup 1.0000x reference)
"""Trainium2 Bass kernel for a 2-layer LSTM + dense + softmax-CE loss.

Model (from the reference):
  B, T, V, E, H = 4096, 80, 80, 8, 256
  x  = emb[features]                  # [B, T, E]
  h1 = LSTM(x;  W1, b1)               # TF BasicLSTMCell, gates (i, j, f, o)
  h2 = LSTM(h1; W2, b2)
  pred = h2[:, -1] @ Wd + bd          # [B, V]
  loss = mean(softmax_xent(pred, labels))

Sharding: pure data parallelism — batch 4096 split 512/core across 8 cores,
weights replicated. Per-core device kernel computes the 512 per-row losses;
host averages the 4096 rows.

Device layout: hidden dim on SBUF partitions, batch (512) on the free dim.
Gate pre-activations are computed as W.T @ [x; h] into PSUM [128, 512] tiles
(8 M-tiles of the 4H=1024 gate dim). fp32r matmuls (full speed at free>=256).
h/c states live in SBUF as 2x[128, 512] f32 tiles and feed the next step's
matmul rhs directly — no transposes anywhere in the loop.
"""

import os
from contextlib import ExitStack

import numpy as np

B, T, V, E, H = 4096, 80, 80, 8, 256
FORGET_BIAS = 1.0
NCORES = 8
BL = B // NCORES          # 512 batch rows per core
NB = BL // 128            # 4 batch tiles of 128 for the loss stage
TG = 16                   # timesteps packed per x-tile partition group
NG = T // TG              # 5 free-dim groups in the packed x tile

_CACHE = {}


def _build_nc(T_steps=T):
    import concourse.tile as tile
    from concourse import bacc, mybir

    f32 = mybir.dt.float32
    f32r = mybir.dt.float32r
    AF = mybir.ActivationFunctionType
    OP = mybir.AluOpType

    nc = bacc.Bacc("TRN2", target_bir_lowering=False, debug=False)

    XT = nc.dram_tensor("XT", [T, E, BL], f32r, kind="ExternalInput")
    OH = nc.dram_tensor("OH", [BL, V], f32, kind="ExternalInput")
    W1 = nc.dram_tensor("W1", [E + H, 4 * H], f32r, kind="ExternalInput")
    W2 = nc.dram_tensor("W2", [2 * H, 4 * H], f32r, kind="ExternalInput")
    B1 = nc.dram_tensor("B1", [128, 8], f32, kind="ExternalInput")
    B2 = nc.dram_tensor("B2", [128, 8], f32, kind="ExternalInput")
    WD = nc.dram_tensor("WD", [H, V], f32r, kind="ExternalInput")
    BD = nc.dram_tensor("BD", [1, V], f32r, kind="ExternalInput")
    LOSS = nc.dram_tensor("LOSS", [NB, 128], f32, kind="ExternalOutput")

    with tile.TileContext(nc) as tc, ExitStack() as ctx:
        wp = ctx.enter_context(tc.tile_pool(name="weights", bufs=1))
        sp = ctx.enter_context(tc.tile_pool(name="state", bufs=1))
        hp = ctx.enter_context(tc.tile_pool(name="h", bufs=2))
        gp = ctx.enter_context(tc.tile_pool(name="gates", bufs=2))
        pp = ctx.enter_context(tc.tile_pool(name="psum", bufs=8, space="PSUM"))
        lp = ctx.enter_context(tc.tile_pool(name="loss", bufs=1))

        # ---- static data loads ----
        w1x = wp.tile([E, 4 * H], f32r, tag="w1x")
        nc.sync.dma_start(w1x[:], W1[0:E, :])
        w1h = []
        for i in range(2):
            t_ = wp.tile([128, 4 * H], f32r, tag=f"w1h{i}")
            nc.sync.dma_start(t_[:], W1[E + 128 * i : E + 128 * (i + 1), :])
            w1h.append(t_)
        w2 = []
        for i in range(4):
            t_ = wp.tile([128, 4 * H], f32r, tag=f"w2{i}")
            nc.sync.dma_start(t_[:], W2[128 * i : 128 * (i + 1), :])
            w2.append(t_)
        b1t = wp.tile([128, 8], f32, tag="b1t")
        nc.sync.dma_start(b1t[:], B1[:])
        b2t = wp.tile([128, 8], f32, tag="b2t")
        nc.sync.dma_start(b2t[:], B2[:])
        wd = []
        for i in range(2):
            t_ = wp.tile([128, V], f32r, tag=f"wd{i}")
            nc.sync.dma_start(t_[:], WD[128 * i : 128 * (i + 1), :])
            wd.append(t_)
        bdt = wp.tile([1, V], f32r, tag="bdt")
        nc.sync.dma_start(bdt[:], BD[:])
        ones_f = wp.tile([1, BL], f32, tag="ones_f")
        nc.vector.memset(ones_f[:], 1.0)
        ones = wp.tile([1, BL], f32r, tag="ones")
        nc.vector.tensor_copy(ones[:], ones_f[:])
        xp = ctx.enter_context(tc.tile_pool(name="xstream", bufs=4))

        # persistent cell states (c starts implicitly at 0 via the t=0 path)
        c1 = [sp.tile([128, BL], f32, tag=f"c1_{s}", name=f"c1_{s}") for s in range(2)]
        c2 = [sp.tile([128, BL], f32, tag=f"c2_{s}", name=f"c2_{s}") for s in range(2)]

        def mm_phase(ps, wchunks, rhs_chunks, start, stop):
            """Emit one K-chunk phase across all 8 M-tiles of a gate group.
            K-major order: consecutive matmuls hit different PSUM banks (so
            they pipeline on the PE) and share the same moving rhs tile."""
            n = len(rhs_chunks)
            for k, (w, rhs) in enumerate(zip(wchunks, rhs_chunks)):
                for m in range(8):
                    ms = slice(128 * m, 128 * (m + 1))
                    nc.tensor.matmul(
                        ps[m][:], w[:, ms], rhs,
                        start=start and k == 0, stop=stop and k == n - 1,
                    )

        def gates(t, layer, ps, bt, c):
            # gate order (i, j, f, o), 2 M-tiles each
            gname = f"L{layer}t"
            sig_i, tanh_j, sig_f, sig_o = [], [], [], []
            for s in range(2):
                ti = gp.tile([128, BL], f32, tag=f"{gname}i{s}")
                nc.scalar.activation(ti[:], ps[s][:], AF.Sigmoid, bias=bt[:, s : s + 1])
                sig_i.append(ti)
                tj = gp.tile([128, BL], f32, tag=f"{gname}j{s}")
                nc.scalar.activation(tj[:], ps[2 + s][:], AF.Tanh, bias=bt[:, 2 + s : 3 + s])
                tanh_j.append(tj)
                if t > 0:
                    tf_ = gp.tile([128, BL], f32, tag=f"{gname}f{s}")
                    nc.scalar.activation(tf_[:], ps[4 + s][:], AF.Sigmoid, bias=bt[:, 4 + s : 5 + s])
                    sig_f.append(tf_)
                to = gp.tile([128, BL], f32, tag=f"{gname}o{s}")
                nc.scalar.activation(to[:], ps[6 + s][:], AF.Sigmoid, bias=bt[:, 6 + s : 7 + s])
                sig_o.append(to)
            hn = []
            for s in range(2):
                if t == 0:
                    # c was 0: c = sig(i) * tanh(j)
                    nc.vector.tensor_tensor(c[s][:], sig_i[s][:], tanh_j[s][:], op=OP.mult)
                else:
                    tmp = gp.tile([128, BL], f32, tag=f"{gname}m{s}")
                    nc.vector.tensor_tensor(tmp[:], sig_i[s][:], tanh_j[s][:], op=OP.mult)
                    nc.vector.tensor_tensor(c[s][:], c[s][:], sig_f[s][:], op=OP.mult)
                    nc.vector.tensor_tensor(c[s][:], c[s][:], tmp[:], op=OP.add)
                th = gp.tile([128, BL], f32, tag=f"{gname}h{s}")
                nc.scalar.activation(th[:], c[s][:], AF.Tanh)
                hnew = hp.tile([128, BL], f32r, tag=f"h{layer}_{s}")
                nc.vector.tensor_tensor(hnew[:], th[:], sig_o[s][:], op=OP.mult)
                hn.append(hnew)
            return hn

        # PE stream per step: L1(t) mm -> L2(t) h2-chunks (cover the h1(t)
        # ACT/DVE chain) -> L2(t) h1-chunks -> L1(t+1) mm (covers the h2(t)
        # chain). No step ever leaves the PE waiting on a gate chain.
        h1 = h2 = None
        for t in range(T_steps):
            xt = xp.tile([E, BL], f32r, tag="xt", name="xt")
            nc.sync.dma_start(xt[:], XT[t])
            ps1 = [pp.tile([128, BL], f32, tag="ps", name=f"ps1_{m}") for m in range(8)]
            if t == 0:
                mm_phase(ps1, [w1x], [xt[:]], start=True, stop=True)
            else:
                mm_phase(ps1, [w1x] + w1h, [xt[:], h1[0][:], h1[1][:]],
                         start=True, stop=True)
            h1 = gates(t, 1, ps1, b1t, c1)
            ps2 = [pp.tile([128, BL], f32, tag="ps", name=f"ps2_{m}") for m in range(8)]
            if t == 0:
                mm_phase(ps2, w2[:2], [h1[0][:], h1[1][:]], start=True, stop=True)
            else:
                mm_phase(ps2, w2[2:], [h2[0][:], h2[1][:]], start=True, stop=False)
                mm_phase(ps2, w2[:2], [h1[0][:], h1[1][:]], start=False, stop=True)
            h2 = gates(t, 2, ps2, b2t, c2)

        # ---- dense + softmax cross-entropy on the last h2 ----
        for m in range(NB):
            ms = slice(128 * m, 128 * (m + 1))
            pd = pp.tile([128, V], mybir.dt.float32, tag="ps", name="pd")
            nc.tensor.matmul(pd[:], h2[0][:, ms], wd[0][:], start=True, stop=False)
            nc.tensor.matmul(pd[:], h2[1][:, ms], wd[1][:], start=False, stop=False)
            nc.tensor.matmul(pd[:], ones[:, ms], bdt[:], start=False, stop=True)

            ohm = lp.tile([128, V], f32, tag=f"oh{m}")
            nc.sync.dma_start(ohm[:], OH[ms, :])

            mx = lp.tile([128, 1], f32, tag=f"mx{m}")
            nc.vector.reduce_max(out=mx[:], in_=pd[:], axis=mybir.AxisListType.X)
            nmx = lp.tile([128, 1], f32, tag=f"nmx{m}")
            nc.vector.tensor_scalar_mul(nmx[:], mx[:], -1.0)
            ex = lp.tile([128, V], f32, tag=f"ex{m}")
            se = lp.tile([128, 1], f32, tag=f"se{m}")
            nc.scalar.activation(ex[:], pd[:], AF.Exp, bias=nmx[:], accum_out=se[:])
            lse = lp.tile([128, 1], f32, tag=f"lse{m}")
            nc.scalar.activation(lse[:], se[:], AF.Ln)
            pk = lp.tile([128, V], f32, tag=f"pk{m}")
            nc.vector.tensor_tensor(pk[:], pd[:], ohm[:], op=OP.mult)
            pks = lp.tile([128, 1], f32, tag=f"pks{m}")
            nc.vector.reduce_sum(out=pks[:], in_=pk[:], axis=mybir.AxisListType.X)
            # loss = max + lse - picked  (lse here is ln(sum exp(pred - max)))
            l0 = lp.tile([128, 1], f32, tag=f"l0{m}")
            nc.vector.tensor_tensor(l0[:], lse[:], pks[:], op=OP.subtract)
            l1_ = lp.tile([128, 1], f32, tag=f"l1{m}")
            nc.vector.tensor_tensor(l1_[:], l0[:], nmx[:], op=OP.subtract)
            nc.sync.dma_start(LOSS[m, :], l1_[:, 0:1])

    nc.compile()
    return nc


def _prep_inputs(features, labels, emb, W1, b1, W2, b2, Wd, bd):
    """Host-side shard + layout prep. Returns in_maps for the 8 cores."""
    features = np.asarray(features)
    labels = np.asarray(labels)
    emb = np.asarray(emb, dtype=np.float32)
    W1 = np.ascontiguousarray(np.asarray(W1, dtype=np.float32))
    W2 = np.ascontiguousarray(np.asarray(W2, dtype=np.float32))
    Wd = np.ascontiguousarray(np.asarray(Wd, dtype=np.float32))

    b1f = np.asarray(b1, dtype=np.float32).copy()
    b1f[2 * H : 3 * H] += FORGET_BIAS
    b2f = np.asarray(b2, dtype=np.float32).copy()
    b2f[2 * H : 3 * H] += FORGET_BIAS
    B1t = np.ascontiguousarray(b1f.reshape(8, 128).T)
    B2t = np.ascontiguousarray(b2f.reshape(8, 128).T)
    BDt = np.ascontiguousarray(np.asarray(bd, dtype=np.float32).reshape(1, V))

    x = emb[features]  # [B, T, E] f32
    eye = np.eye(V, dtype=np.float32)

    in_maps = []
    for c in range(NCORES):
        sl = slice(c * BL, (c + 1) * BL)
        xc = np.ascontiguousarray(x[sl].transpose(1, 2, 0))  # [T, E, BL]
        oh = eye[labels[sl]]
        in_maps.append({
            "XT": xc, "OH": np.ascontiguousarray(oh),
            "W1": W1, "W2": W2, "B1": B1t, "B2": B2t,
            "WD": Wd, "BD": BDt,
        })
    return in_maps


def _run(inputs, trace=False, **spmd_kwargs):
    from concourse.bass_utils import run_bass_kernel_spmd

    if "nc" not in _CACHE:
        _CACHE["nc"] = _build_nc()
    nc = _CACHE["nc"]
    in_maps = _prep_inputs(**inputs)
    res = run_bass_kernel_spmd(
        nc, in_maps, list(range(NCORES)), trace=trace, **spmd_kwargs
    )
    rows = np.concatenate([np.asarray(r["LOSS"], np.float64).ravel() for r in res.results])
    loss = np.asarray(rows.mean(), dtype=np.float32)
    return loss, res


def kernel(**inputs):
    loss, _ = _run(inputs, trace=False)
    return loss


# revision 17
# speedup vs baseline: 1.3755x; 1.3755x over previous
"""Trainium2 Bass kernel for a 2-layer LSTM + dense + softmax-CE loss.

Model (from the reference):
  B, T, V, E, H = 4096, 80, 80, 8, 256
  x  = emb[features]                  # [B, T, E]
  h1 = LSTM(x;  W1, b1)               # TF BasicLSTMCell, gates (i, j, f, o)
  h2 = LSTM(h1; W2, b2)
  pred = h2[:, -1] @ Wd + bd          # [B, V]
  loss = mean(softmax_xent(pred, labels))

Sharding: pure data parallelism — batch 4096 split 512/core across 8 cores,
weights replicated. Per-core device kernel computes the 512 per-row losses;
host averages the 4096 rows.

Device layout: hidden dim on SBUF partitions, batch (512) on the free dim.
Gate pre-activations are computed as W.T @ [x; h] into PSUM [128, 512] tiles
(8 M-tiles of the 4H=1024 gate dim). fp32r matmuls (full speed at free>=256).
h/c states live in SBUF as 2x[128, 512] f32 tiles and feed the next step's
matmul rhs directly — no transposes anywhere in the loop.
"""

import os
from contextlib import ExitStack

import numpy as np

B, T, V, E, H = 4096, 80, 80, 8, 256
FORGET_BIAS = 1.0
NCORES = 8
BL = B // NCORES          # 512 batch rows per core
NB = BL // 128            # 4 batch tiles of 128 for the loss stage
TG = 16                   # timesteps packed per x-tile partition group
NG = T // TG              # 5 free-dim groups in the packed x tile

_CACHE = {}


def _build_nc(T_steps=T):
    import concourse.tile as tile
    from concourse import bacc, mybir

    f32 = mybir.dt.float32
    f32r = mybir.dt.float32r
    AF = mybir.ActivationFunctionType
    OP = mybir.AluOpType

    nc = bacc.Bacc("TRN2", target_bir_lowering=False, debug=False)

    XT = nc.dram_tensor("XT", [T, E, BL], f32r, kind="ExternalInput")
    OH = nc.dram_tensor("OH", [BL, V], f32, kind="ExternalInput")
    W1 = nc.dram_tensor("W1", [E + H, 4 * H], f32r, kind="ExternalInput")
    W2 = nc.dram_tensor("W2", [2 * H, 4 * H], f32r, kind="ExternalInput")
    B1 = nc.dram_tensor("B1", [128, 8], f32, kind="ExternalInput")
    B2 = nc.dram_tensor("B2", [128, 8], f32, kind="ExternalInput")
    WD = nc.dram_tensor("WD", [H, V], f32r, kind="ExternalInput")
    BD = nc.dram_tensor("BD", [1, V], f32r, kind="ExternalInput")
    LOSS = nc.dram_tensor("LOSS", [NB, 128], f32, kind="ExternalOutput")

    with tile.TileContext(nc) as tc, ExitStack() as ctx:
        wp = ctx.enter_context(tc.tile_pool(name="weights", bufs=1))
        sp = ctx.enter_context(tc.tile_pool(name="state", bufs=1))
        hp = ctx.enter_context(tc.tile_pool(name="h", bufs=2))
        gp = ctx.enter_context(tc.tile_pool(name="gates", bufs=2))
        pp = ctx.enter_context(tc.tile_pool(name="psum", bufs=8, space="PSUM"))
        lp = ctx.enter_context(tc.tile_pool(name="loss", bufs=1))

        # ---- static data loads ----
        w1x = wp.tile([E, 4 * H], f32r, tag="w1x")
        nc.sync.dma_start(w1x[:], W1[0:E, :])
        w1h = []
        for i in range(2):
            t_ = wp.tile([128, 4 * H], f32r, tag=f"w1h{i}")
            nc.sync.dma_start(t_[:], W1[E + 128 * i : E + 128 * (i + 1), :])
            w1h.append(t_)
        w2 = []
        for i in range(4):
            t_ = wp.tile([128, 4 * H], f32r, tag=f"w2{i}")
            nc.sync.dma_start(t_[:], W2[128 * i : 128 * (i + 1), :])
            w2.append(t_)
        b1t = wp.tile([128, 8], f32, tag="b1t")
        nc.sync.dma_start(b1t[:], B1[:])
        b2t = wp.tile([128, 8], f32, tag="b2t")
        nc.sync.dma_start(b2t[:], B2[:])
        wd = []
        for i in range(2):
            t_ = wp.tile([128, V], f32r, tag=f"wd{i}")
            nc.sync.dma_start(t_[:], WD[128 * i : 128 * (i + 1), :])
            wd.append(t_)
        bdt = wp.tile([1, V], f32r, tag="bdt")
        nc.sync.dma_start(bdt[:], BD[:])
        ones_f = wp.tile([1, BL], f32, tag="ones_f")
        nc.vector.memset(ones_f[:], 1.0)
        ones = wp.tile([1, BL], f32r, tag="ones")
        nc.vector.tensor_copy(ones[:], ones_f[:])
        xp = ctx.enter_context(tc.tile_pool(name="xstream", bufs=4))

        # persistent cell states (c starts implicitly at 0 via the t=0 path)
        c1 = [sp.tile([128, BL], f32, tag=f"c1_{s}", name=f"c1_{s}") for s in range(2)]
        c2 = [sp.tile([128, BL], f32, tag=f"c2_{s}", name=f"c2_{s}") for s in range(2)]

        def mm_phase(ps, wchunks, rhs_chunks, start, stop):
            """Emit one K-chunk phase across all 8 M-tiles of a gate group."""
            for m in range(8):
                ms = slice(128 * m, 128 * (m + 1))
                n = len(rhs_chunks)
                for k, (w, rhs) in enumerate(zip(wchunks, rhs_chunks)):
                    nc.tensor.matmul(
                        ps[m][:], w[:, ms], rhs,
                        start=start and k == 0, stop=stop and k == n - 1,
                    )

        def gates(t, layer, ps, bt, c):
            # gate order (i, j, f, o), 2 M-tiles each
            gname = f"L{layer}t"
            sig_i, tanh_j, sig_f, sig_o = [], [], [], []
            for s in range(2):
                ti = gp.tile([128, BL], f32, tag=f"{gname}i{s}")
                nc.scalar.activation(ti[:], ps[s][:], AF.Sigmoid, bias=bt[:, s : s + 1])
                sig_i.append(ti)
                tj = gp.tile([128, BL], f32, tag=f"{gname}j{s}")
                nc.scalar.activation(tj[:], ps[2 + s][:], AF.Tanh, bias=bt[:, 2 + s : 3 + s])
                tanh_j.append(tj)
                if t > 0:
                    tf_ = gp.tile([128, BL], f32, tag=f"{gname}f{s}")
                    nc.scalar.activation(tf_[:], ps[4 + s][:], AF.Sigmoid, bias=bt[:, 4 + s : 5 + s])
                    sig_f.append(tf_)
                to = gp.tile([128, BL], f32, tag=f"{gname}o{s}")
                nc.scalar.activation(to[:], ps[6 + s][:], AF.Sigmoid, bias=bt[:, 6 + s : 7 + s])
                sig_o.append(to)
            hn = []
            for s in range(2):
                if t == 0:
                    # c was 0: c = sig(i) * tanh(j)
                    nc.vector.tensor_tensor(c[s][:], sig_i[s][:], tanh_j[s][:], op=OP.mult)
                else:
                    tmp = gp.tile([128, BL], f32, tag=f"{gname}m{s}")
                    nc.vector.tensor_tensor(tmp[:], sig_i[s][:], tanh_j[s][:], op=OP.mult)
                    nc.vector.tensor_tensor(c[s][:], c[s][:], sig_f[s][:], op=OP.mult)
                    nc.vector.tensor_tensor(c[s][:], c[s][:], tmp[:], op=OP.add)
                th = gp.tile([128, BL], f32, tag=f"{gname}h{s}")
                nc.scalar.activation(th[:], c[s][:], AF.Tanh)
                hnew = hp.tile([128, BL], f32r, tag=f"h{layer}_{s}")
                nc.vector.tensor_tensor(hnew[:], th[:], sig_o[s][:], op=OP.mult)
                hn.append(hnew)
            return hn

        # PE stream per step: L1(t) mm -> L2(t) h2-chunks (cover the h1(t)
        # ACT/DVE chain) -> L2(t) h1-chunks -> L1(t+1) mm (covers the h2(t)
        # chain). No step ever leaves the PE waiting on a gate chain.
        h1 = h2 = None
        for t in range(T_steps):
            xt = xp.tile([E, BL], f32r, tag="xt", name="xt")
            nc.sync.dma_start(xt[:], XT[t])
            ps1 = [pp.tile([128, BL], f32, tag="ps", name=f"ps1_{m}") for m in range(8)]
            if t == 0:
                mm_phase(ps1, [w1x], [xt[:]], start=True, stop=True)
            else:
                # x (K=8) last: its short stream can't hide a 128-row
                # weight load, so let the K=128 chunks lead.
                mm_phase(ps1, w1h + [w1x], [h1[0][:], h1[1][:], xt[:]],
                         start=True, stop=True)
            h1 = gates(t, 1, ps1, b1t, c1)
            ps2 = [pp.tile([128, BL], f32, tag="ps", name=f"ps2_{m}") for m in range(8)]
            if t == 0:
                mm_phase(ps2, w2[:2], [h1[0][:], h1[1][:]], start=True, stop=True)
            else:
                mm_phase(ps2, w2[2:], [h2[0][:], h2[1][:]], start=True, stop=False)
                mm_phase(ps2, w2[:2], [h1[0][:], h1[1][:]], start=False, stop=True)
            h2 = gates(t, 2, ps2, b2t, c2)

        # ---- dense + softmax cross-entropy on the last h2 ----
        for m in range(NB):
            ms = slice(128 * m, 128 * (m + 1))
            pd = pp.tile([128, V], mybir.dt.float32, tag="ps", name="pd")
            nc.tensor.matmul(pd[:], h2[0][:, ms], wd[0][:], start=True, stop=False)
            nc.tensor.matmul(pd[:], h2[1][:, ms], wd[1][:], start=False, stop=False)
            nc.tensor.matmul(pd[:], ones[:, ms], bdt[:], start=False, stop=True)

            ohm = lp.tile([128, V], f32, tag=f"oh{m}")
            nc.sync.dma_start(ohm[:], OH[ms, :])

            mx = lp.tile([128, 1], f32, tag=f"mx{m}")
            nc.vector.reduce_max(out=mx[:], in_=pd[:], axis=mybir.AxisListType.X)
            nmx = lp.tile([128, 1], f32, tag=f"nmx{m}")
            nc.vector.tensor_scalar_mul(nmx[:], mx[:], -1.0)
            ex = lp.tile([128, V], f32, tag=f"ex{m}")
            se = lp.tile([128, 1], f32, tag=f"se{m}")
            nc.scalar.activation(ex[:], pd[:], AF.Exp, bias=nmx[:], accum_out=se[:])
            lse = lp.tile([128, 1], f32, tag=f"lse{m}")
            nc.scalar.activation(lse[:], se[:], AF.Ln)
            pk = lp.tile([128, V], f32, tag=f"pk{m}")
            nc.vector.tensor_tensor(pk[:], pd[:], ohm[:], op=OP.mult)
            pks = lp.tile([128, 1], f32, tag=f"pks{m}")
            nc.vector.reduce_sum(out=pks[:], in_=pk[:], axis=mybir.AxisListType.X)
            # loss = max + lse - picked  (lse here is ln(sum exp(pred - max)))
            l0 = lp.tile([128, 1], f32, tag=f"l0{m}")
            nc.vector.tensor_tensor(l0[:], lse[:], pks[:], op=OP.subtract)
            l1_ = lp.tile([128, 1], f32, tag=f"l1{m}")
            nc.vector.tensor_tensor(l1_[:], l0[:], nmx[:], op=OP.subtract)
            nc.sync.dma_start(LOSS[m, :], l1_[:, 0:1])

    nc.compile()
    return nc


def _prep_inputs(features, labels, emb, W1, b1, W2, b2, Wd, bd):
    """Host-side shard + layout prep. Returns in_maps for the 8 cores."""
    features = np.asarray(features)
    labels = np.asarray(labels)
    emb = np.asarray(emb, dtype=np.float32)
    W1 = np.ascontiguousarray(np.asarray(W1, dtype=np.float32))
    W2 = np.ascontiguousarray(np.asarray(W2, dtype=np.float32))
    Wd = np.ascontiguousarray(np.asarray(Wd, dtype=np.float32))

    b1f = np.asarray(b1, dtype=np.float32).copy()
    b1f[2 * H : 3 * H] += FORGET_BIAS
    b2f = np.asarray(b2, dtype=np.float32).copy()
    b2f[2 * H : 3 * H] += FORGET_BIAS
    B1t = np.ascontiguousarray(b1f.reshape(8, 128).T)
    B2t = np.ascontiguousarray(b2f.reshape(8, 128).T)
    BDt = np.ascontiguousarray(np.asarray(bd, dtype=np.float32).reshape(1, V))

    x = emb[features]  # [B, T, E] f32
    eye = np.eye(V, dtype=np.float32)

    in_maps = []
    for c in range(NCORES):
        sl = slice(c * BL, (c + 1) * BL)
        xc = np.ascontiguousarray(x[sl].transpose(1, 2, 0))  # [T, E, BL]
        oh = eye[labels[sl]]
        in_maps.append({
            "XT": xc, "OH": np.ascontiguousarray(oh),
            "W1": W1, "W2": W2, "B1": B1t, "B2": B2t,
            "WD": Wd, "BD": BDt,
        })
    return in_maps


def _run(inputs, trace=False, **spmd_kwargs):
    from concourse.bass_utils import run_bass_kernel_spmd

    if "nc" not in _CACHE:
        _CACHE["nc"] = _build_nc()
    nc = _CACHE["nc"]
    in_maps = _prep_inputs(**inputs)
    res = run_bass_kernel_spmd(
        nc, in_maps, list(range(NCORES)), trace=trace, **spmd_kwargs
    )
    rows = np.concatenate([np.asarray(r["LOSS"], np.float64).ravel() for r in res.results])
    loss = np.asarray(rows.mean(), dtype=np.float32)
    return loss, res


def kernel(**inputs):
    loss, _ = _run(inputs, trace=False)
    return loss


# revision 18
# speedup vs baseline: 1.5457x; 1.1238x over previous
"""Trainium2 Bass kernel for a 2-layer LSTM + dense + softmax-CE loss.

Model (from the reference):
  B, T, V, E, H = 4096, 80, 80, 8, 256
  x  = emb[features]                  # [B, T, E]
  h1 = LSTM(x;  W1, b1)               # TF BasicLSTMCell, gates (i, j, f, o)
  h2 = LSTM(h1; W2, b2)
  pred = h2[:, -1] @ Wd + bd          # [B, V]
  loss = mean(softmax_xent(pred, labels))

Sharding: pure data parallelism — batch 4096 split 512/core across 8 cores,
weights replicated. Per-core device kernel computes the 512 per-row losses;
host averages the 4096 rows.

Device layout: hidden dim on SBUF partitions, batch (512) on the free dim.
Gate pre-activations are computed as W.T @ [x; h] into PSUM [128, 512] tiles
(8 M-tiles of the 4H=1024 gate dim). fp32r matmuls (full speed at free>=256).
h/c states live in SBUF as 2x[128, 512] f32 tiles and feed the next step's
matmul rhs directly — no transposes anywhere in the loop.
"""

import os
from contextlib import ExitStack

import numpy as np

B, T, V, E, H = 4096, 80, 80, 8, 256
FORGET_BIAS = 1.0
NCORES = 8
BL = B // NCORES          # 512 batch rows per core
NB = BL // 128            # 4 batch tiles of 128 for the loss stage
TG = 16                   # timesteps packed per x-tile partition group
NG = T // TG              # 5 free-dim groups in the packed x tile

_CACHE = {}


def _build_nc(T_steps=T):
    import concourse.tile as tile
    from concourse import bacc, mybir

    f32 = mybir.dt.float32
    f32r = mybir.dt.bfloat16  # matmul-path dtype
    AF = mybir.ActivationFunctionType
    OP = mybir.AluOpType

    nc = bacc.Bacc("TRN2", target_bir_lowering=False, debug=False)

    XT = nc.dram_tensor("XT", [T, E, BL], f32r, kind="ExternalInput")
    OH = nc.dram_tensor("OH", [BL, V], f32, kind="ExternalInput")
    W1 = nc.dram_tensor("W1", [E + H, 4 * H], f32r, kind="ExternalInput")
    W2 = nc.dram_tensor("W2", [2 * H, 4 * H], f32r, kind="ExternalInput")
    B1 = nc.dram_tensor("B1", [128, 8], f32, kind="ExternalInput")
    B2 = nc.dram_tensor("B2", [128, 8], f32, kind="ExternalInput")
    WD = nc.dram_tensor("WD", [H, V], f32r, kind="ExternalInput")
    BD = nc.dram_tensor("BD", [1, V], f32r, kind="ExternalInput")
    LOSS = nc.dram_tensor("LOSS", [NB, 128], f32, kind="ExternalOutput")

    with tile.TileContext(nc) as tc, ExitStack() as ctx:
        wp = ctx.enter_context(tc.tile_pool(name="weights", bufs=1))
        sp = ctx.enter_context(tc.tile_pool(name="state", bufs=1))
        hp = ctx.enter_context(tc.tile_pool(name="h", bufs=2))
        gp = ctx.enter_context(tc.tile_pool(name="gates", bufs=2))
        pp = ctx.enter_context(tc.tile_pool(name="psum", bufs=8, space="PSUM"))
        lp = ctx.enter_context(tc.tile_pool(name="loss", bufs=1))

        # ---- static data loads ----
        w1x = wp.tile([E, 4 * H], f32r, tag="w1x")
        nc.sync.dma_start(w1x[:], W1[0:E, :])
        w1h = []
        for i in range(2):
            t_ = wp.tile([128, 4 * H], f32r, tag=f"w1h{i}")
            nc.sync.dma_start(t_[:], W1[E + 128 * i : E + 128 * (i + 1), :])
            w1h.append(t_)
        w2 = []
        for i in range(4):
            t_ = wp.tile([128, 4 * H], f32r, tag=f"w2{i}")
            nc.sync.dma_start(t_[:], W2[128 * i : 128 * (i + 1), :])
            w2.append(t_)
        b1t = wp.tile([128, 8], f32, tag="b1t")
        nc.sync.dma_start(b1t[:], B1[:])
        b2t = wp.tile([128, 8], f32, tag="b2t")
        nc.sync.dma_start(b2t[:], B2[:])
        wd = []
        for i in range(2):
            t_ = wp.tile([128, V], f32r, tag=f"wd{i}")
            nc.sync.dma_start(t_[:], WD[128 * i : 128 * (i + 1), :])
            wd.append(t_)
        bdt = wp.tile([1, V], f32r, tag="bdt")
        nc.sync.dma_start(bdt[:], BD[:])
        ones_f = wp.tile([1, BL], f32, tag="ones_f")
        nc.vector.memset(ones_f[:], 1.0)
        ones = wp.tile([1, BL], f32r, tag="ones")
        nc.vector.tensor_copy(ones[:], ones_f[:])
        xp = ctx.enter_context(tc.tile_pool(name="xstream", bufs=4))

        # persistent cell states (c starts implicitly at 0 via the t=0 path)
        c1 = [sp.tile([128, BL], f32, tag=f"c1_{s}", name=f"c1_{s}") for s in range(2)]
        c2 = [sp.tile([128, BL], f32, tag=f"c2_{s}", name=f"c2_{s}") for s in range(2)]

        def mm_phase(ps, wchunks, rhs_chunks, start, stop):
            """Emit one K-chunk phase across all 8 M-tiles of a gate group."""
            for m in range(8):
                ms = slice(128 * m, 128 * (m + 1))
                n = len(rhs_chunks)
                for k, (w, rhs) in enumerate(zip(wchunks, rhs_chunks)):
                    nc.tensor.matmul(
                        ps[m][:], w[:, ms], rhs,
                        start=start and k == 0, stop=stop and k == n - 1,
                    )

        def gates(t, layer, ps, bt, c):
            # gate order (i, j, f, o), 2 M-tiles each
            gname = f"L{layer}t"
            sig_i, tanh_j, sig_f, sig_o = [], [], [], []
            for s in range(2):
                ti = gp.tile([128, BL], f32, tag=f"{gname}i{s}")
                nc.scalar.activation(ti[:], ps[s][:], AF.Sigmoid, bias=bt[:, s : s + 1])
                sig_i.append(ti)
                tj = gp.tile([128, BL], f32, tag=f"{gname}j{s}")
                nc.scalar.activation(tj[:], ps[2 + s][:], AF.Tanh, bias=bt[:, 2 + s : 3 + s])
                tanh_j.append(tj)
                if t > 0:
                    tf_ = gp.tile([128, BL], f32, tag=f"{gname}f{s}")
                    nc.scalar.activation(tf_[:], ps[4 + s][:], AF.Sigmoid, bias=bt[:, 4 + s : 5 + s])
                    sig_f.append(tf_)
                to = gp.tile([128, BL], f32, tag=f"{gname}o{s}")
                nc.scalar.activation(to[:], ps[6 + s][:], AF.Sigmoid, bias=bt[:, 6 + s : 7 + s])
                sig_o.append(to)
            hn = []
            for s in range(2):
                if t == 0:
                    # c was 0: c = sig(i) * tanh(j)
                    nc.vector.tensor_tensor(c[s][:], sig_i[s][:], tanh_j[s][:], op=OP.mult)
                else:
                    tmp = gp.tile([128, BL], f32, tag=f"{gname}m{s}")
                    nc.vector.tensor_tensor(tmp[:], sig_i[s][:], tanh_j[s][:], op=OP.mult)
                    nc.vector.tensor_tensor(c[s][:], c[s][:], sig_f[s][:], op=OP.mult)
                    nc.vector.tensor_tensor(c[s][:], c[s][:], tmp[:], op=OP.add)
                th = gp.tile([128, BL], f32, tag=f"{gname}h{s}")
                nc.scalar.activation(th[:], c[s][:], AF.Tanh)
                hnew = hp.tile([128, BL], f32r, tag=f"h{layer}_{s}")
                nc.vector.tensor_tensor(hnew[:], th[:], sig_o[s][:], op=OP.mult)
                hn.append(hnew)
            return hn

        # PE stream per step: L1(t) mm -> L2(t) h2-chunks (cover the h1(t)
        # ACT/DVE chain) -> L2(t) h1-chunks -> L1(t+1) mm (covers the h2(t)
        # chain). No step ever leaves the PE waiting on a gate chain.
        h1 = h2 = None
        for t in range(T_steps):
            xt = xp.tile([E, BL], f32r, tag="xt", name="xt")
            nc.sync.dma_start(xt[:], XT[t])
            ps1 = [pp.tile([128, BL], f32, tag="ps", name=f"ps1_{m}") for m in range(8)]
            if t == 0:
                mm_phase(ps1, [w1x], [xt[:]], start=True, stop=True)
            else:
                # x (K=8) last: its short stream can't hide a 128-row
                # weight load, so let the K=128 chunks lead.
                mm_phase(ps1, w1h + [w1x], [h1[0][:], h1[1][:], xt[:]],
                         start=True, stop=True)
            h1 = gates(t, 1, ps1, b1t, c1)
            ps2 = [pp.tile([128, BL], f32, tag="ps", name=f"ps2_{m}") for m in range(8)]
            if t == 0:
                mm_phase(ps2, w2[:2], [h1[0][:], h1[1][:]], start=True, stop=True)
            else:
                mm_phase(ps2, w2[2:], [h2[0][:], h2[1][:]], start=True, stop=False)
                mm_phase(ps2, w2[:2], [h1[0][:], h1[1][:]], start=False, stop=True)
            h2 = gates(t, 2, ps2, b2t, c2)

        # ---- dense + softmax cross-entropy on the last h2 ----
        for m in range(NB):
            ms = slice(128 * m, 128 * (m + 1))
            pd = pp.tile([128, V], mybir.dt.float32, tag="ps", name="pd")
            nc.tensor.matmul(pd[:], h2[0][:, ms], wd[0][:], start=True, stop=False)
            nc.tensor.matmul(pd[:], h2[1][:, ms], wd[1][:], start=False, stop=False)
            nc.tensor.matmul(pd[:], ones[:, ms], bdt[:], start=False, stop=True)

            ohm = lp.tile([128, V], f32, tag=f"oh{m}")
            nc.sync.dma_start(ohm[:], OH[ms, :])

            mx = lp.tile([128, 1], f32, tag=f"mx{m}")
            nc.vector.reduce_max(out=mx[:], in_=pd[:], axis=mybir.AxisListType.X)
            nmx = lp.tile([128, 1], f32, tag=f"nmx{m}")
            nc.vector.tensor_scalar_mul(nmx[:], mx[:], -1.0)
            ex = lp.tile([128, V], f32, tag=f"ex{m}")
            se = lp.tile([128, 1], f32, tag=f"se{m}")
            nc.scalar.activation(ex[:], pd[:], AF.Exp, bias=nmx[:], accum_out=se[:])
            lse = lp.tile([128, 1], f32, tag=f"lse{m}")
            nc.scalar.activation(lse[:], se[:], AF.Ln)
            pk = lp.tile([128, V], f32, tag=f"pk{m}")
            nc.vector.tensor_tensor(pk[:], pd[:], ohm[:], op=OP.mult)
            pks = lp.tile([128, 1], f32, tag=f"pks{m}")
            nc.vector.reduce_sum(out=pks[:], in_=pk[:], axis=mybir.AxisListType.X)
            # loss = max + lse - picked  (lse here is ln(sum exp(pred - max)))
            l0 = lp.tile([128, 1], f32, tag=f"l0{m}")
            nc.vector.tensor_tensor(l0[:], lse[:], pks[:], op=OP.subtract)
            l1_ = lp.tile([128, 1], f32, tag=f"l1{m}")
            nc.vector.tensor_tensor(l1_[:], l0[:], nmx[:], op=OP.subtract)
            nc.sync.dma_start(LOSS[m, :], l1_[:, 0:1])

    nc.compile()
    return nc


def _prep_inputs(features, labels, emb, W1, b1, W2, b2, Wd, bd):
    """Host-side shard + layout prep. Returns in_maps for the 8 cores."""
    import ml_dtypes

    bf16 = ml_dtypes.bfloat16
    features = np.asarray(features)
    labels = np.asarray(labels)
    emb = np.asarray(emb, dtype=np.float32)
    W1 = np.ascontiguousarray(np.asarray(W1, dtype=np.float32).astype(bf16))
    W2 = np.ascontiguousarray(np.asarray(W2, dtype=np.float32).astype(bf16))
    Wd = np.ascontiguousarray(np.asarray(Wd, dtype=np.float32).astype(bf16))

    b1f = np.asarray(b1, dtype=np.float32).copy()
    b1f[2 * H : 3 * H] += FORGET_BIAS
    b2f = np.asarray(b2, dtype=np.float32).copy()
    b2f[2 * H : 3 * H] += FORGET_BIAS
    B1t = np.ascontiguousarray(b1f.reshape(8, 128).T)
    B2t = np.ascontiguousarray(b2f.reshape(8, 128).T)
    BDt = np.ascontiguousarray(np.asarray(bd, dtype=np.float32).reshape(1, V).astype(bf16))

    x = emb[features]  # [B, T, E] f32
    eye = np.eye(V, dtype=np.float32)

    in_maps = []
    for c in range(NCORES):
        sl = slice(c * BL, (c + 1) * BL)
        xc = np.ascontiguousarray(x[sl].transpose(1, 2, 0).astype(bf16))  # [T, E, BL]
        oh = eye[labels[sl]]
        in_maps.append({
            "XT": xc, "OH": np.ascontiguousarray(oh),
            "W1": W1, "W2": W2, "B1": B1t, "B2": B2t,
            "WD": Wd, "BD": BDt,
        })
    return in_maps


def _run(inputs, trace=False, **spmd_kwargs):
    from concourse.bass_utils import run_bass_kernel_spmd

    if "nc" not in _CACHE:
        _CACHE["nc"] = _build_nc()
    nc = _CACHE["nc"]
    in_maps = _prep_inputs(**inputs)
    res = run_bass_kernel_spmd(
        nc, in_maps, list(range(NCORES)), trace=trace, **spmd_kwargs
    )
    rows = np.concatenate([np.asarray(r["LOSS"], np.float64).ravel() for r in res.results])
    loss = np.asarray(rows.mean(), dtype=np.float32)
    return loss, res


def kernel(**inputs):
    loss, _ = _run(inputs, trace=False)
    return loss
